# revision 81
# baseline (speedup 1.0000x reference)
"""Trainium2 Bass kernel for the Kruskal (CP/Tucker) linear layer.

Math: the reference reconstructs W (4096x4096) from a rank-16 CP core and
Tucker factors, then computes y = x @ W.T + bias.  Because the 6D core is a
CP (Kruskal) tensor of rank 16, W itself is exactly rank 16:

    W = g_out @ g_in.T
    g_in[def, r]  = (f3@c3)[d,r] * (f4@c4)[e,r] * (f5@c5)[f,r]   (4096 x 16)
    g_out[abc, r] = (f0@c0)[a,r] * (f1@c1)[b,r] * (f2@c2)[c,r]   (4096 x 16)

so  y = (x @ g_in) @ g_out.T + bias.  The device kernel computes the two
x-dependent projections; the tiny factor-only products (g_in/g_out, ~100
KFLOP) are prepared on the host.

Sharding: data-parallel over the batch (4096 rows -> 8 cores x 512). No
collectives.  The host ships each core its x slice PRE-TRANSPOSED and cast
to bf16 (x^T slice, 4096 features x 512 batch).  Feature-major HBM layout
means stage 1 needs no on-device transpose at all: the contraction dim
lands on partitions straight off the DMA.  Per core:
  1. 8 HWDGE loads of x^T k-groups (128, 4, 512) bf16
  2. stage 1: 32 accumulating matmuls  t^T(16,512) += g_in_kt.T @ x^T_kt
     (N=512, one PSUM bank for the whole core's t^T)
  3. DVE copy t^T -> SBUF bf16 (rank rows + ones row for the bias)
  4. stage 2: 32 bf16 matmuls (4 batch tiles x 8 col tiles, N=512)
     y = [t,1] @ [g_out.T; bias]
  5. DVE/ACT copy PSUM->SBUF, DMA y fp32 out per batch tile
"""

import numpy as np
import ml_dtypes

N_CORES = 8
BATCH = 4096
D = 4096          # in/out features (16*16*16)
R = 16            # CP rank
P = 128           # partitions
NB = BATCH // N_CORES   # 512 batch rows per core
BT = NB // P            # 4 batch tiles per core
KT = D // P             # 32 feature k-tiles
GS = [4, 10, 10, 8]                # k-tiles per DMA load group (sums to KT).
                                   # Fewer, fatter loads: each load is 128
                                   # descriptors regardless of width, and
                                   # load completions are gated by one DMA
                                   # engine's descriptor FIFO — fewer loads
                                   # means its backlog drains sooner.  The
                                   # PE warm-up covers the later first-group
                                   # completion.
NT = 512                # output column tile (PSUM bank / max moving size)
JT = D // NT            # 8 output column tiles
CW = KT * R + NB        # const-pack width: gin columns + aux columns

_PROGRAM = None


def _build_program():
    import concourse.tile as tile
    from concourse import bacc, mybir

    nc = bacc.Bacc(
        "TRN2",
        target_bir_lowering=False,
        debug=False,
        enable_asserts=False,
        num_devices=N_CORES,
    )
    # x^T slice for this core: feature-major k-tile-major bf16, host-packed:
    # xflat[p, kt*NB + b] = x[b, kt*128 + p].  Loaded in column-range groups
    # (GS k-tiles each) so every load is 128 descriptors of contiguous rows.
    # Group 0 is shipped separately with gin packed in front of its columns,
    # so the single first DMA delivers both gin and the first k-tiles.
    xg0_d = nc.dram_tensor(
        "xg0c", (P, KT * R + GS[0] * NB), mybir.dt.bfloat16, kind="ExternalInput"
    )
    xT_d = nc.dram_tensor(
        "xrc", (P, (KT - GS[0]) * NB), mybir.dt.bfloat16, kind="ExternalInput"
    )
    gout_d = nc.dram_tensor("goutT", (R + 1, D), mybir.dt.bfloat16, kind="ExternalInput")
    # aux: init image of t^T (rows 0..15 zeros, row 16 ones for the bias)
    aux_d = nc.dram_tensor("aux", (R + 1, NB), mybir.dt.bfloat16, kind="ExternalInput")
    # y leaves the device as bf16 (the host upcasts to fp32): halves the
    # 8.4 MB/core write-back — the largest single block at the HBM write
    # cap — for ~0.2% rms extra rounding against a 2e-2 error budget
    y_d = nc.dram_tensor("yc", (NB, D), mybir.dt.bfloat16, kind="ExternalOutput")

    with tile.TileContext(nc) as tc:
        with (
            tc.tile_pool(name="const", bufs=1) as constp,
            tc.tile_pool(name="xT", bufs=len(GS)) as xTp,
            tc.tile_pool(name="ysb", bufs=4) as ysbp,
            tc.tile_pool(name="tpsum", bufs=1, space="PSUM") as tpsump,
            tc.tile_pool(name="ypsum", bufs=3, space="PSUM") as ypsump,
            tc.tile_pool(name="wmps", bufs=1, space="PSUM") as wmpsp,
        ):
            # x^T loads alternate the two HWDGE queues (HWDGE queue depth is
            # ~4; tiny/late loads beyond that are fine since their data is
            # needed late).  gin rides inside group 0; gout/aux ride at the
            # sync queue's tail (stage-2 data).  No SWDGE anywhere: its
            # software descriptor path crawls and poisons the engines.
            xg0 = xTp.tile([P, KT * R + GS[0] * NB], mybir.dt.bfloat16)
            nc.scalar.dma_start(xg0[:], xg0_d.ap())
            gin_sb = xg0[:, 0 : KT * R]
            # per-group SBUF tiles; group g covers k-tiles kt0[g]..kt0[g+1]
            xT_sb = [xg0[:, KT * R :]]
            off = 0
            for ng in range(1, len(GS)):
                xt = xTp.tile([P, GS[ng] * NB], mybir.dt.bfloat16)
                # group 0 is on scalar, so odd groups go to sync: consecutive
                # consumption-order groups load on opposite queues in
                # parallel (group 1 sharing group 0's queue serialized them
                # and stalled stage 1 ~2us at the first boundary)
                eng = (nc.scalar, nc.sync)[ng % 2]
                eng.dma_start(xt[:], xT_d.ap()[:, off * NB : (off + GS[ng]) * NB])
                xT_sb.append(xt)
                off += GS[ng]
            gout_sb = constp.tile([R + 1, D], mybir.dt.bfloat16)
            nc.sync.dma_start(gout_sb[:], gout_d.ap())
            tT_sb = constp.tile([R + 1, NB], mybir.dt.bfloat16)
            nc.sync.dma_start(tT_sb[:], aux_d.ap())

            # PE warm-up: the tensor engine only reaches its fast p-state
            # after ~3us of continuous work, and the first x load doesn't
            # complete until ~13-18us (fixed preamble + DMA-engine semaphore
            # backlog).  A chain of dummy matmuls on a zeroed scratch tile
            # into a dead PSUM bank keeps the PE busy through that window so
            # stage 1 starts at full clock.  No data deps beyond the memset.
            wm_sb = constp.tile([P, NT], mybir.dt.bfloat16)
            nc.vector.memset(wm_sb[:], 0)
            # 16 matmuls span ~8-13us; longer chains (42, ending ~19.5us)
            # measured reproducibly ~4us slower: they delay stage-1 dispatch
            # when x lands first, and x-group arrival stalls drop the p-state
            # again regardless.
            wm_ps = wmpsp.tile([P, NT], mybir.dt.float32)
            NWM = 18
            for w in range(NWM):
                nc.tensor.matmul(
                    wm_ps[:],
                    lhsT=wm_sb[:, 0:P],
                    rhs=wm_sb[:],
                    start=(w == 0),
                    stop=(w == NWM - 1),
                    skip_group_check=True,
                )

            # stage 1: all 32 k-tiles accumulate into one PSUM tile, but as
            # one start/stop group PER x-load group.  The tile scheduler
            # treats each accumulation group as a unit whose dependencies are
            # the union of its inputs, so one 32-matmul group would stall
            # until the entire x load finished; per-load groups start as
            # their x lands.  PSUM accumulation is per-write on HW, so
            # chaining groups with start=False is exact.
            tT_ps = tpsump.tile([R, NB], mybir.dt.float32)
            kt = 0
            for ng in range(len(GS)):
                for g in range(GS[ng]):
                    nc.tensor.matmul(
                        tT_ps[:],
                        lhsT=gin_sb[:, kt * R : (kt + 1) * R],
                        rhs=xT_sb[ng][:, g * NB : (g + 1) * NB],
                        start=(kt == 0),
                        stop=(g == GS[ng] - 1),
                        skip_group_check=True,
                    )
                    kt += 1
            # t^T rows 0..15 = (x@g_in).T, cast bf16 (row 16 = ones via aux);
            # halves split across DVE and ACT so the cast runs in parallel
            nc.vector.tensor_copy(tT_sb[0:R, 0 : NB // 2], tT_ps[:, 0 : NB // 2])
            nc.scalar.copy(tT_sb[0:R, NB // 2 :], tT_ps[:, NB // 2 :])

            # stage 2: y = [t,1] @ [g_out.T; bias], per batch tile.
            # jt pairs share a 2-bank PSUM tile; one 1024-col copy per pair,
            # rotated across DVE / ACT / Pool to keep the PE from stalling
            cp = 0
            for bt in range(BT):
                y_sb = ysbp.tile([P, D], mybir.dt.bfloat16)
                for jp in range(JT // 2):
                    y_ps = ypsump.tile([P, 2 * NT], mybir.dt.float32)
                    for h in range(2):
                        jt = jp * 2 + h
                        nc.tensor.matmul(
                            y_ps[:, h * NT : (h + 1) * NT],
                            lhsT=tT_sb[:, bt * P : (bt + 1) * P],
                            rhs=gout_sb[:, jt * NT : (jt + 1) * NT],
                        )
                    dst = y_sb[:, jp * 2 * NT : (jp + 1) * 2 * NT]
                    if cp % 2 == 0:
                        nc.vector.tensor_copy(dst, y_ps[:])
                    else:
                        nc.scalar.copy(dst, y_ps[:])
                    cp += 1
                # two half-row stores per batch tile (the first fires as soon
                # as the first two copy pairs land), all on the sync queue:
                # the scalar queue carries ACT copies during stage 2 and a
                # store trigger there would sit behind them
                for yh in range(2):
                    nc.sync.dma_start(
                        y_d.ap()[bt * P : (bt + 1) * P, yh * (D // 2) : (yh + 1) * (D // 2)],
                        y_sb[:, yh * (D // 2) : (yh + 1) * (D // 2)],
                    )

    nc.compile()
    return nc


def _get_program():
    global _PROGRAM
    if _PROGRAM is None:
        _PROGRAM = _build_program()
    return _PROGRAM


def _host_factors(inputs):
    """Build g_in (SBUF layout), [g_out.T; bias], t^T init image (bf16)."""
    c = [np.asarray(inputs[f"c{i}"], dtype=np.float64) for i in range(6)]
    f = [np.asarray(inputs[f"f{i}"], dtype=np.float64) for i in range(6)]
    bias = np.asarray(inputs["bias"], dtype=np.float32)
    h = [f[i] @ c[i] for i in range(6)]  # (16,16) each
    g_out = (
        h[0][:, None, None, :] * h[1][None, :, None, :] * h[2][None, None, :, :]
    ).reshape(D, R)
    g_in = (
        h[3][:, None, None, :] * h[4][None, :, None, :] * h[5][None, None, :, :]
    ).reshape(D, R)
    # gin SBUF layout: gin_l[p, kt*R + r] = g_in[kt*128 + p, r]
    gin_l = np.ascontiguousarray(
        g_in.reshape(KT, P, R).transpose(1, 0, 2).reshape(P, KT * R)
    ).astype(ml_dtypes.bfloat16)
    goutT = np.concatenate(
        [g_out.T.astype(np.float32), bias[None, :]], axis=0
    ).astype(ml_dtypes.bfloat16)  # (17, 4096)
    aux = np.zeros((R + 1, NB), dtype=ml_dtypes.bfloat16)
    aux[R, :] = 1.0
    return gin_l, goutT, aux


# test-harness hooks (unused in graded path)
TRACE = False
LAST_RESULTS = None


def kernel(**inputs):
    from concourse.bass_utils import run_bass_kernel_spmd

    global LAST_RESULTS
    x = np.asarray(inputs["x"], dtype=np.float32)
    # host-side: cast to bf16, transpose to feature-major, k-tile-major:
    # xall[ci][p, kt*NB + b] = x[ci*NB + b, kt*128 + p]
    xb = x.astype(ml_dtypes.bfloat16)  # (BATCH, D)
    xall = np.ascontiguousarray(
        xb.reshape(N_CORES, NB, KT, P).transpose(0, 3, 2, 1)
    ).reshape(N_CORES, P, KT * NB)
    gin_l, goutT, aux = _host_factors(inputs)
    # group 0 with gin packed in front of its first GS[0] k-tiles
    g0w = GS[0] * NB
    xg0 = np.empty((N_CORES, P, KT * R + g0w), dtype=ml_dtypes.bfloat16)
    xg0[:, :, 0 : KT * R] = gin_l[None]
    xg0[:, :, KT * R :] = xall[:, :, 0:g0w]
    nc = _get_program()
    in_maps = [
        {
            "xg0c": xg0[ci],
            "xrc": np.ascontiguousarray(xall[ci, :, g0w:]),
            "goutT": goutT,
            "aux": aux,
        }
        for ci in range(N_CORES)
    ]
    res = run_bass_kernel_spmd(
        nc, in_maps, core_ids=list(range(N_CORES)), trace=TRACE
    )
    LAST_RESULTS = res
    # device returns bf16; upcast to the contract dtype on the host
    y = np.concatenate([np.asarray(r["yc"]) for r in res.results], axis=0)
    return np.ascontiguousarray(y.astype(np.float32))


if __name__ == "__main__":
    # quick smoke test with random data
    rng = np.random.default_rng(0)
    ins = {"x": rng.normal(size=(BATCH, D)).astype(np.float32)}
    for i in range(6):
        ins[f"c{i}"] = (rng.normal(size=(8, 16)) * 0.1).astype(np.float32)
        ins[f"f{i}"] = (rng.normal(size=(16, 8)) * 0.1).astype(np.float32)
    ins["bias"] = np.zeros(D, dtype=np.float32)
    y = kernel(**ins)
    print("y", y.shape, y.dtype)
